# revision 10
# baseline (speedup 1.0000x reference)
"""Trainium2 Bass kernel: multi-head attention with Toeplitz relative bias.

Problem: B=16, L=1024, F=512, H=8, D=64 ViT patch attention.
Sharding: data-parallel over batch, 2 batches per core across 8 cores.

Device-side design (per core, fully unrolled Tile program):
  - Host pre-transposes inputs to xT [F, L] (bf16); 1/sqrt(D) folded into Wq.
  - qT/kT computed transposed ([fout, L]) in natural head-pair layout (head h
    on partitions (h%2)*64..+64 of fout chunk h//2). No zero padding: the
    scores matmul runs K=64 with both operands at partition base (h%2)*64
    (PE quadrant addressing).
  - v computed natural [L, fout] and packed as vA [128, kt, h, 65] fp16 with
    a ones column at index 64 (bv folded in via a ones-row matmul).
  - Scores [k, q] per (b, h, kt) in PSUM; ACT does exp (the only ACT work:
    ACT is the pipeline floor at ~143us); DVE/Pool multiply the host-exp'd
    Toeplitz bias in fp16 (DVE 2x mode; split to keep both engines even).
  - attn@v with vA stationary: U[0:64] = unnormalized x^T, U[64] = softmax
    denominator, accumulated over the 8 kt chunks in PSUM. Streams 512-col
    moving operands so ldweights (~97ns fixed) hide behind 213ns streams
    (the old ex-stationary design paid ~100us of exposed ldweights).
  - Normalization: rc = 1/denom (DVE divide), partition_broadcast to 64 rows
    (GpSimd Q7), fused multiply+cast -> xatT bf16 (DVE), writing odd heads at
    partition base 64 so the output projection gets K=128 chunks.
  - Output projection from xatT (+bo via ones-row matmul), Pool copies PSUM
    ->SBUF, DMA out.
  - No max-subtraction in softmax: |scores| <~ 2 by construction.
"""

import sys

import numpy as np

for _p in ("/opt/trn_rl_repo",):
    if _p not in sys.path:
        sys.path.insert(0, _p)

import ml_dtypes

import concourse.bass as bass
import concourse.mybir as mybir
import concourse.tile as tile
from concourse import bacc
from concourse.bass_utils import run_bass_kernel_spmd

B, L, F, H, D = 16, 1024, 512, 8, 64
NX, NY = 32, 32
NCORES = 8
BPC = B // NCORES  # batches per core
FP32 = mybir.dt.float32
F32R = mybir.dt.float32r
BF16 = mybir.dt.bfloat16
FP16 = mybir.dt.float16
Exp = mybir.ActivationFunctionType.Exp
Add = mybir.AluOpType.add
Mult = mybir.AluOpType.mult
Div = mybir.AluOpType.divide

# how many of the 8 bias-multiply kt-tiles per (b,h) go to DVE (rest: Pool).
# GPSIMD/Pool cannot touch PSUM, so every PSUM-reading op lives on DVE/ACT;
# Pool gets SBUF-only work (bias multiplies, partition broadcast, memsets).
DVE_MULT_KT = 5


def _build():
    nc = bacc.Bacc("TRN2", target_bir_lowering=False, debug=False)

    xqT_d = nc.dram_tensor("xqT", [BPC, F, L], BF16, kind="ExternalInput").ap()
    xkvT_d = nc.dram_tensor("xkvT", [BPC, F, L], BF16, kind="ExternalInput").ap()
    Wq_d = nc.dram_tensor("Wq", [F, F], BF16, kind="ExternalInput").ap()
    Wk_d = nc.dram_tensor("Wk", [F, F], BF16, kind="ExternalInput").ap()
    Wv_d = nc.dram_tensor("Wv", [F, F], BF16, kind="ExternalInput").ap()
    Wo_d = nc.dram_tensor("Wo", [F, F], BF16, kind="ExternalInput").ap()
    bq_d = nc.dram_tensor("bq", [F], FP32, kind="ExternalInput").ap()
    bk_d = nc.dram_tensor("bk", [F], FP32, kind="ExternalInput").ap()
    bv_d = nc.dram_tensor("bv", [128, F], F32R, kind="ExternalInput").ap()
    bo_d = nc.dram_tensor("bo", [128, F], F32R, kind="ExternalInput").ap()
    biasT_d = nc.dram_tensor("biasT", [H, L, L], FP16, kind="ExternalInput").ap()
    ones_d = nc.dram_tensor("ones", [128, 128], F32R, kind="ExternalInput").ap()
    out_d = nc.dram_tensor("out", [BPC, L, F], FP32, kind="ExternalOutput").ap()

    with tile.TileContext(nc) as tc:
        with (
            tc.tile_pool(name="const", bufs=1) as cpool,
            tc.tile_pool(name="xin", bufs=2) as xpool,
            tc.tile_pool(name="qkv", bufs=2) as qpool,
            tc.tile_pool(name="bias", bufs=2) as bpool,
            tc.tile_pool(name="es", bufs=3) as espool,
            tc.tile_pool(name="exq", bufs=5) as epool,
            tc.tile_pool(name="nrm", bufs=2) as npool,
            tc.tile_pool(name="os", bufs=2) as opool,
            tc.tile_pool(name="psA", bufs=2, space="PSUM") as psA,
            tc.tile_pool(name="psU", bufs=1, space="PSUM") as psU,
        ):
            # ---- constants ----
            Wv_s = cpool.tile([128, 4 * F], BF16, tag="Wv")
            Wq_s = cpool.tile([128, 4 * F], BF16, tag="Wq")
            Wk_s = cpool.tile([128, 4 * F], BF16, tag="Wk")
            Wo_s = cpool.tile([128, 4 * F], BF16, tag="Wo")

            def load_w(w_s, w_d):
                nc.sync.dma_start(
                    out=w_s[:].rearrange("p (c n) -> p c n", c=4),
                    in_=w_d.rearrange("(c p) n -> p c n", c=4),
                )

            for kc in range(4):  # stream Wv first so v-proj starts ASAP
                nc.sync.dma_start(
                    out=Wv_s[:, kc * F : (kc + 1) * F],
                    in_=Wv_d[kc * 128 : (kc + 1) * 128, :],
                )
            ones_s = cpool.tile([128, 128], F32R, tag="ones")
            nc.sync.dma_start(out=ones_s[:], in_=ones_d)
            bv_s = cpool.tile([128, F], F32R, tag="bv")
            nc.sync.dma_start(out=bv_s[:], in_=bv_d)

            # ---- per-batch tiles ----
            qT, kT, vA, xatT, xq, xkv = [], [], [], [], [], []
            for b in range(BPC):
                xq.append(xpool.tile([128, 4 * L], BF16, tag="xq", name=f"xq{b}"))
                xkv.append(xpool.tile([128, 4 * L], BF16, tag="xkv", name=f"xkv{b}"))
                qT.append(qpool.tile([128, 4 * L], BF16, tag="qT", name=f"qT{b}"))
                kT.append(qpool.tile([128, 4 * L], BF16, tag="kT", name=f"kT{b}"))
                vA.append(
                    qpool.tile([128, 8 * 8 * 65], FP16, tag="vA", name=f"vA{b}")
                )
                xatT.append(
                    qpool.tile([128, 4 * L], BF16, tag="xatT", name=f"xatT{b}")
                )

            # ---- phase A: load inputs + v projection ----
            for b in range(BPC):
                for lq in range(4):
                    nc.sync.dma_start(
                        out=xkv[b][:]
                        .rearrange("p (c l) -> p c l", c=4)[
                            :, :, lq * 256 : (lq + 1) * 256
                        ],
                        in_=xkvT_d[b].rearrange("(c p) l -> p c l", c=4)[
                            :, :, lq * 256 : (lq + 1) * 256
                        ],
                    )
                if b == 0:
                    load_w(Wq_s, Wq_d)
                    load_w(Wk_s, Wk_d)
                    bq_s = cpool.tile([128, 4], FP32, tag="bq")
                    bk_s = cpool.tile([128, 4], FP32, tag="bk")
                    for b_s, b_d in ((bq_s, bq_d), (bk_s, bk_d)):
                        nc.sync.dma_start(
                            out=b_s[:], in_=b_d.rearrange("(c p) -> p c", p=128)
                        )
                    load_w(Wo_s, Wo_d)
                    bo_s = cpool.tile([128, F], F32R, tag="bo")
                    nc.sync.dma_start(out=bo_s[:], in_=bo_d)

                # v natural (+bv via ones-row matmul): xT stationary, Wv moving
                vA_v = vA[b][:].rearrange("p (t h w) -> p t h w", t=8, h=8)
                for lt in range(8):
                    pv = psA.tile([128, 512], FP32, tag="psv")
                    for kc in range(4):
                        nc.tensor.matmul(
                            pv[:],
                            xkv[b][:, kc * L + lt * 128 : kc * L + (lt + 1) * 128],
                            Wv_s[:, kc * F : (kc + 1) * F],
                            start=(kc == 0),
                            stop=False,
                        )
                    nc.tensor.matmul(pv[:], ones_s[:], bv_s[:], start=False, stop=True)
                    nc.vector.tensor_copy(
                        vA_v[:, lt, :, 0:64],
                        pv[:].rearrange("p (h w) -> p h w", h=8),
                    )
                nc.gpsimd.memset(vA_v[:, :, :, 64:65], 1.0)
                nc.sync.dma_start(
                    out=xq[b][:].rearrange("p (c l) -> p c l", c=4),
                    in_=xqT_d[b].rearrange("(c p) l -> p c l", c=4),
                )

            def qk_proj(fo):
                # q/k projections for fout chunk fo (= head pair 2fo, 2fo+1),
                # emitted just-in-time before the heads that consume them.
                for b in range(BPC):
                    for which, w_s, b_s, x_t, dst in (
                        ("q", Wq_s, bq_s, xq[b], qT[b]),
                        ("k", Wk_s, bk_s, xkv[b], kT[b]),
                    ):
                        for lc in range(2):
                            pq = psA.tile([128, 512], FP32, tag="psv")
                            for kc in range(4):
                                nc.tensor.matmul(
                                    pq[:],
                                    w_s[:, kc * F + fo * 128 : kc * F + (fo + 1) * 128],
                                    x_t[:, kc * L + lc * 512 : kc * L + (lc + 1) * 512],
                                    start=(kc == 0),
                                    stop=(kc == 3),
                                )
                            nc.vector.tensor_scalar(
                                dst[:, fo * L + lc * 512 : fo * L + (lc + 1) * 512],
                                pq[:],
                                b_s[:, fo : fo + 1],
                                None,
                                op0=Add,
                            )

            def emit_out(b):
                # output projection (+bo via ones-row matmul)
                for lt in range(8):
                    po = psA.tile([128, 512], FP32, tag="psv")
                    for c in range(4):
                        nc.tensor.matmul(
                            po[:],
                            xatT[b][:, c * L + lt * 128 : c * L + (lt + 1) * 128],
                            Wo_s[:, c * F : (c + 1) * F],
                            start=(c == 0),
                            stop=False,
                        )
                    nc.tensor.matmul(po[:], ones_s[:], bo_s[:], start=False, stop=True)
                    os_t = opool.tile([128, 512], FP32, tag="os")
                    nc.scalar.copy(os_t[:], po[:])
                    nc.sync.dma_start(
                        out=out_d[b, lt * 128 : (lt + 1) * 128, :], in_=os_t[:]
                    )

            # ---- phase B: attention head loop ----
            for h in range(H):
                if h % 2 == 0:
                    qk_proj(h // 2)
                hp = (h % 2) * 64  # partition base within fout chunk
                hc = (h // 2) * L  # column base of fout chunk
                bias_t = bpool.tile([128, 8 * L], FP16, tag="bias")
                for kt in range(8):
                    nc.sync.dma_start(
                        out=bias_t[:, kt * L : (kt + 1) * L],
                        in_=biasT_d[h, kt * 128 : (kt + 1) * 128, :],
                    )
                for b in range(BPC):
                    # scores (K=64 quadrant matmuls) -> exp -> bias multiply
                    ex_tiles = []
                    for kt in range(8):
                        ps = psA.tile([128, L], FP32, tag="pss")
                        for qc in range(2):
                            nc.tensor.matmul(
                                ps[:, qc * 512 : (qc + 1) * 512],
                                kT[b][
                                    hp : hp + 64, hc + kt * 128 : hc + (kt + 1) * 128
                                ],
                                qT[b][hp : hp + 64, hc + qc * 512 : hc + (qc + 1) * 512],
                                start=True,
                                stop=True,
                            )
                        es = espool.tile([128, L], FP16, tag="es")
                        nc.scalar.activation(es[:], ps[:], Exp)
                        ex = epool.tile([128, L], FP16, tag="ex")
                        eng = nc.vector if kt < DVE_MULT_KT else nc.gpsimd
                        eng.tensor_tensor(
                            ex[:], es[:], bias_t[:, kt * L : (kt + 1) * L], Mult
                        )
                        ex_tiles.append(ex)

                    # attn @ v_aug, vA stationary: U[0:64]=x^T, U[64]=denom
                    U = psU.tile([128, L], FP32, tag="u")
                    vA_v = vA[b][:].rearrange("p (t h w) -> p t h w", t=8, h=8)
                    for kt in range(8):
                        for qc in range(2):
                            nc.tensor.matmul(
                                U[0:65, qc * 512 : (qc + 1) * 512],
                                vA_v[:, kt, h, :],
                                ex_tiles[kt][:, qc * 512 : (qc + 1) * 512],
                                start=(kt == 0),
                                stop=(kt == 7),
                            )

                    # normalize: xatT[hp:hp+64, hc..] = U[0:64] * (1/denom)
                    rc = npool.tile([1, L], FP32, tag="rc")
                    nc.vector.reciprocal(rc[:], U[64:65, :])
                    rcb = npool.tile([64, L], FP32, tag="rcb")
                    nc.gpsimd.partition_broadcast(rcb[:], rc[:], channels=64)
                    nc.vector.tensor_tensor(
                        xatT[b][hp : hp + 64, hc : hc + L], U[0:64, :], rcb[:], Mult
                    )
                    if h == H - 1:
                        emit_out(b)

    nc.compile()
    return nc


_NC = None


def _get_nc():
    global _NC
    if _NC is None:
        _NC = _build()
    return _NC


def _prep_in_maps(inputs):
    bf16 = ml_dtypes.bfloat16
    xq = np.asarray(inputs["inputs_q"], dtype=np.float32)
    xkv = np.asarray(inputs["inputs_kv"], dtype=np.float32)
    Wq = (np.asarray(inputs["Wq"], dtype=np.float32) * 0.125).astype(bf16)
    bq = np.asarray(inputs["bq"], dtype=np.float32) * 0.125
    Wk = np.asarray(inputs["Wk"], dtype=np.float32).astype(bf16)
    bk = np.asarray(inputs["bk"], dtype=np.float32)
    Wv = np.asarray(inputs["Wv"], dtype=np.float32).astype(bf16)
    bv_pad = np.zeros((128, F), dtype=np.float32)
    bv_pad[0] = np.asarray(inputs["bv"], dtype=np.float32)
    Wo = np.asarray(inputs["Wo"], dtype=np.float32).astype(bf16)
    bo_pad = np.zeros((128, F), dtype=np.float32)
    bo_pad[0] = np.asarray(inputs["bo"], dtype=np.float32)
    onesrow = np.zeros((128, 128), dtype=np.float32)
    onesrow[0] = 1.0
    toe = np.asarray(inputs["toeplitz"], dtype=np.float32)

    xqT = np.ascontiguousarray(xq.transpose(0, 2, 1)).astype(bf16)  # [B, F, L]
    xkvT = np.ascontiguousarray(xkv.transpose(0, 2, 1)).astype(bf16)

    coords = np.arange(L)
    xi, yi = coords // NY, coords % NY
    dx = xi[:, None] - xi[None, :] + NX
    dy = yi[:, None] - yi[None, :] + NY
    idx = dx * (2 * NY) + dy  # [L(q), L(k)]
    bias = toe[:, idx]  # [H, L(q), L(k)]
    biasT = np.exp(np.ascontiguousarray(bias.transpose(0, 2, 1))).astype(np.float16)

    in_maps = []
    for i in range(NCORES):
        sl = slice(i * BPC, (i + 1) * BPC)
        in_maps.append(
            {
                "xqT": np.ascontiguousarray(xqT[sl]),
                "xkvT": np.ascontiguousarray(xkvT[sl]),
                "Wq": Wq, "Wk": Wk, "Wv": Wv, "Wo": Wo,
                "bq": bq, "bk": bk, "bv": bv_pad, "bo": bo_pad,
                "biasT": biasT,
                "ones": onesrow,
            }
        )
    return in_maps


def _run(inputs, trace=False):
    from concourse.bass_interp import get_hw_module

    nc = _get_nc()
    in_maps = _prep_in_maps(inputs)
    old_m = nc.m
    nc.m = get_hw_module(nc.m)
    try:
        res = run_bass_kernel_spmd(
            nc, in_maps, core_ids=list(range(NCORES)), trace=trace
        )
    finally:
        nc.m = old_m
    out = np.concatenate([r["out"] for r in res.results], axis=0)  # [B, L, F]
    return out.reshape(B, L, H, D), res


def kernel(**inputs) -> np.ndarray:
    out, _ = _run(inputs, trace=False)
    return out


# revision 24
# speedup vs baseline: 1.5401x; 1.5401x over previous
"""Trainium2 Bass kernel: multi-head attention with Toeplitz relative bias.

Problem: B=16, L=1024, F=512, H=8, D=64 ViT patch attention.
Sharding: data-parallel over batch, 2 batches per core across 8 cores.

Device-side design (per core, fully unrolled Tile program):
  - Host pre-transposes inputs to xT [F, L] (bf16); 1/sqrt(D) folded into Wq.
  - qT/kT computed transposed ([fout, L]) in natural head-pair layout (head h
    on partitions (h%2)*64..+64 of fout chunk h//2). No zero padding: the
    scores matmul runs K=64 with both operands at partition base (h%2)*64
    (PE quadrant addressing).
  - v computed natural [L, fout] and packed as vA [128, kt, h, 65] fp16 with
    a ones column at index 64 (bv folded in via a ones-row matmul).
  - Scores [k, q] per (b, h, kt) in PSUM; ACT does exp (the only ACT work:
    ACT is the pipeline floor at ~143us); DVE/Pool multiply the host-exp'd
    Toeplitz bias in fp16 (DVE 2x mode; split to keep both engines even).
  - attn@v with vA stationary: U[0:64] = unnormalized x^T, U[64] = softmax
    denominator, accumulated over the 8 kt chunks in PSUM. Streams 512-col
    moving operands so ldweights (~97ns fixed) hide behind 213ns streams
    (the old ex-stationary design paid ~100us of exposed ldweights).
  - Normalization: rc = 1/denom (DVE divide), partition_broadcast to 64 rows
    (GpSimd Q7), fused multiply+cast -> xatT bf16 (DVE), writing odd heads at
    partition base 64 so the output projection gets K=128 chunks.
  - Output projection from xatT (+bo via ones-row matmul), Pool copies PSUM
    ->SBUF, DMA out.
  - No max-subtraction in softmax: |scores| <~ 2 by construction.
"""

import sys

import numpy as np

for _p in ("/opt/trn_rl_repo",):
    if _p not in sys.path:
        sys.path.insert(0, _p)

import ml_dtypes

import concourse.bass as bass
import concourse.mybir as mybir
import concourse.tile as tile
from concourse import bacc
from concourse.bass_utils import run_bass_kernel_spmd

B, L, F, H, D = 16, 1024, 512, 8, 64
NX, NY = 32, 32
NCORES = 8
BPC = B // NCORES  # batches per core
FP32 = mybir.dt.float32
F32R = mybir.dt.float32r
BF16 = mybir.dt.bfloat16
FP16 = mybir.dt.float16
Exp = mybir.ActivationFunctionType.Exp
Add = mybir.AluOpType.add
Mult = mybir.AluOpType.mult
Div = mybir.AluOpType.divide

# how many of the 8 bias-multiply kt-tiles per (b,h) go to DVE (rest: Pool).
# GPSIMD/Pool cannot touch PSUM, so every PSUM-reading op lives on DVE/ACT;
# Pool gets SBUF-only work (bias multiplies, partition broadcast, memsets).
DVE_MULT_KT = 5


def _build():
    nc = bacc.Bacc("TRN2", target_bir_lowering=False, debug=False)

    xqT_d = nc.dram_tensor("xqT", [BPC, F, L], BF16, kind="ExternalInput").ap()
    xkvT_d = nc.dram_tensor("xkvT", [BPC, F, L], BF16, kind="ExternalInput").ap()
    Wq_d = nc.dram_tensor("Wq", [F, F], BF16, kind="ExternalInput").ap()
    Wk_d = nc.dram_tensor("Wk", [F, F], BF16, kind="ExternalInput").ap()
    Wv_d = nc.dram_tensor("Wv", [F, F], BF16, kind="ExternalInput").ap()
    Wo_d = nc.dram_tensor("Wo", [F, F], BF16, kind="ExternalInput").ap()
    bq_d = nc.dram_tensor("bq", [F], FP32, kind="ExternalInput").ap()
    bk_d = nc.dram_tensor("bk", [F], FP32, kind="ExternalInput").ap()
    bv_d = nc.dram_tensor("bv", [128, F], F32R, kind="ExternalInput").ap()
    bo_d = nc.dram_tensor("bo", [128, F], F32R, kind="ExternalInput").ap()
    biasT_d = nc.dram_tensor("biasT", [H, L, L], FP16, kind="ExternalInput").ap()
    ones_d = nc.dram_tensor("ones", [128, 128], F32R, kind="ExternalInput").ap()
    out_d = nc.dram_tensor("out", [BPC, L, F], FP32, kind="ExternalOutput").ap()
    # DRAM scratch for the reciprocal rows: partition-broadcast DMA reads
    # require a DRAM source (SBUF sources need nonzero partition step)
    rcd_d = nc.dram_tensor("rcd", [BPC, 8, L], FP16, kind="Internal").ap()

    with tile.TileContext(nc) as tc:
        with (
            tc.tile_pool(name="const", bufs=1) as cpool,
            tc.tile_pool(name="xin", bufs=2) as xpool,
            tc.tile_pool(name="qkv", bufs=2) as qpool,
            tc.tile_pool(name="bias", bufs=2) as bpool,
            tc.tile_pool(name="es", bufs=3) as espool,
            tc.tile_pool(name="exq", bufs=5) as epool,
            tc.tile_pool(name="nrm", bufs=1) as npool,
            tc.tile_pool(name="os", bufs=2) as opool,
            tc.tile_pool(name="psA", bufs=2, space="PSUM") as psA,
            tc.tile_pool(name="psU", bufs=1, space="PSUM") as psU,
        ):
            # ---- constants ----
            Wv_s = cpool.tile([128, 4 * F], BF16, tag="Wv")
            Wq_s = cpool.tile([128, 4 * F], BF16, tag="Wq")
            Wk_s = cpool.tile([128, 4 * F], BF16, tag="Wk")
            Wo_s = cpool.tile([128, 4 * F], BF16, tag="Wo")

            def load_w(w_s, w_d):
                nc.sync.dma_start(
                    out=w_s[:].rearrange("p (c n) -> p c n", c=4),
                    in_=w_d.rearrange("(c p) n -> p c n", c=4),
                )

            for kc in range(4):  # stream Wv first so v-proj starts ASAP
                nc.sync.dma_start(
                    out=Wv_s[:, kc * F : (kc + 1) * F],
                    in_=Wv_d[kc * 128 : (kc + 1) * 128, :],
                )
            ones_s = cpool.tile([128, 128], F32R, tag="ones")
            nc.sync.dma_start(out=ones_s[:], in_=ones_d)
            bv_s = cpool.tile([128, F], F32R, tag="bv")
            nc.sync.dma_start(out=bv_s[:], in_=bv_d)

            # ---- per-batch tiles ----
            qT, kT, vA, xatT, xq, xkv, dstage = [], [], [], [], [], [], []
            for b in range(BPC):
                xq.append(xpool.tile([128, 4 * L], BF16, tag="xq", name=f"xq{b}"))
                xkv.append(xpool.tile([128, 4 * L], BF16, tag="xkv", name=f"xkv{b}"))
                qT.append(qpool.tile([128, 4 * L], BF16, tag="qT", name=f"qT{b}"))
                kT.append(qpool.tile([128, 4 * L], BF16, tag="kT", name=f"kT{b}"))
                vA.append(
                    qpool.tile([128, 8 * 8 * 65], FP16, tag="vA", name=f"vA{b}")
                )
                xatT.append(
                    qpool.tile([128, 4 * L], BF16, tag="xatT", name=f"xatT{b}")
                )
                dstage.append(
                    qpool.tile([128, 2 * L], FP16, tag="dstage", name=f"dstage{b}")
                )

            # ---- phase A: load inputs + v projection ----
            for b in range(BPC):
                for lq in range(4):
                    nc.sync.dma_start(
                        out=xkv[b][:]
                        .rearrange("p (c l) -> p c l", c=4)[
                            :, :, lq * 256 : (lq + 1) * 256
                        ],
                        in_=xkvT_d[b].rearrange("(c p) l -> p c l", c=4)[
                            :, :, lq * 256 : (lq + 1) * 256
                        ],
                    )
                if b == 0:
                    load_w(Wq_s, Wq_d)
                    load_w(Wk_s, Wk_d)
                    bq_s = cpool.tile([128, 4], FP32, tag="bq")
                    bk_s = cpool.tile([128, 4], FP32, tag="bk")
                    for b_s, b_d in ((bq_s, bq_d), (bk_s, bk_d)):
                        nc.sync.dma_start(
                            out=b_s[:], in_=b_d.rearrange("(c p) -> p c", p=128)
                        )
                    load_w(Wo_s, Wo_d)
                    bo_s = cpool.tile([128, F], F32R, tag="bo")
                    nc.sync.dma_start(out=bo_s[:], in_=bo_d)

                # v natural (+bv via ones-row matmul): xT stationary, Wv moving
                vA_v = vA[b][:].rearrange("p (t h w) -> p t h w", t=8, h=8)
                for lt in range(8):
                    pv = psA.tile([128, 512], FP32, tag="psv")
                    for kc in range(4):
                        nc.tensor.matmul(
                            pv[:],
                            xkv[b][:, kc * L + lt * 128 : kc * L + (lt + 1) * 128],
                            Wv_s[:, kc * F : (kc + 1) * F],
                            start=(kc == 0),
                            stop=False,
                        )
                    nc.tensor.matmul(pv[:], ones_s[:], bv_s[:], start=False, stop=True)
                    nc.scalar.copy(
                        vA_v[:, lt, :, 0:64],
                        pv[:].rearrange("p (h w) -> p h w", h=8),
                    )
                nc.gpsimd.memset(vA_v[:, :, :, 64:65], 1.0)
                nc.sync.dma_start(
                    out=xq[b][:].rearrange("p (c l) -> p c l", c=4),
                    in_=xqT_d[b].rearrange("(c p) l -> p c l", c=4),
                )

            def qk_proj(fo):
                # q/k projections for fout chunk fo (= head pair 2fo, 2fo+1),
                # emitted just-in-time before the heads that consume them.
                for b in range(BPC):
                    for which, w_s, b_s, x_t, dst in (
                        ("q", Wq_s, bq_s, xq[b], qT[b]),
                        ("k", Wk_s, bk_s, xkv[b], kT[b]),
                    ):
                        for lc in range(2):
                            pq = psA.tile([128, 512], FP32, tag="psv")
                            for kc in range(4):
                                nc.tensor.matmul(
                                    pq[:],
                                    w_s[:, kc * F + fo * 128 : kc * F + (fo + 1) * 128],
                                    x_t[:, kc * L + lc * 512 : kc * L + (lc + 1) * 512],
                                    start=(kc == 0),
                                    stop=(kc == 3),
                                )
                            nc.vector.tensor_scalar(
                                dst[:, fo * L + lc * 512 : fo * L + (lc + 1) * 512],
                                pq[:],
                                b_s[:, fo : fo + 1],
                                None,
                                op0=Add,
                            )

            def emit_norm_out(b):
                # batched softmax normalization. Engine partition starts are
                # restricted to {0,32,64,96}, so: DMA-repack the 8 quadrant
                # denominator rows to 8 contiguous rows, ONE [8, L] reciprocal
                # (DVE reciprocal costs ~6.4ns/column regardless of partition
                # count), cast to fp16, DMA-spread back to quadrant rows for
                # the GpSimd broadcasts, then in-place multiplies (DVE 2x).
                d8 = npool.tile([8, L], FP16, tag="d8")
                nc.sync.dma_start(
                    out=d8[:],
                    in_=dstage[b][:].rearrange("p (c l) -> p c l", c=2)[
                        0:97:32, :, :
                    ],
                )
                rc32 = npool.tile([8, L], FP32, tag="rc32")
                nc.vector.reciprocal(rc32[:], d8[:])
                rc16 = npool.tile([8, L], FP16, tag="rc16")
                nc.vector.tensor_copy(rc16[:], rc32[:])
                nc.sync.dma_start(out=rcd_d[b], in_=rc16[:])
                for h in range(H):
                    hp = (h % 2) * 64
                    hc = (h // 2) * L
                    # broadcast 1/denom to 64 rows via DMA from DRAM (stride-0
                    # partition source); land at xatT's partition base so the
                    # two SBUF inputs of tensor_tensor share a base.
                    # d8 pack order is partition-major: row = (h%4)*2 + h//4
                    hr = (h % 4) * 2 + h // 4
                    rcb = npool.tile([128, L], FP16, tag="rcb", bufs=2)
                    nc.sync.dma_start(
                        out=rcb[hp : hp + 64, :],
                        in_=rcd_d[b][hr : hr + 1, :].to_broadcast((64, L)),
                    )
                    nc.vector.tensor_tensor(
                        xatT[b][hp : hp + 64, hc : hc + L],
                        xatT[b][hp : hp + 64, hc : hc + L],
                        rcb[hp : hp + 64, :],
                        Mult,
                    )
                    del rcb
                # output projection (+bo via ones-row matmul)
                for lt in range(8):
                    po = psA.tile([128, 512], FP32, tag="psv")
                    for c in range(4):
                        nc.tensor.matmul(
                            po[:],
                            xatT[b][:, c * L + lt * 128 : c * L + (lt + 1) * 128],
                            Wo_s[:, c * F : (c + 1) * F],
                            start=(c == 0),
                            stop=False,
                        )
                    nc.tensor.matmul(po[:], ones_s[:], bo_s[:], start=False, stop=True)
                    os_t = opool.tile([128, 512], FP32, tag="os")
                    nc.scalar.copy(os_t[:], po[:])
                    nc.sync.dma_start(
                        out=out_d[b, lt * 128 : (lt + 1) * 128, :], in_=os_t[:]
                    )

            # ---- phase B: attention head loop ----
            for h in range(H):
                if h % 2 == 0:
                    qk_proj(h // 2)
                hp = (h % 2) * 64  # partition base within fout chunk
                hc = (h // 2) * L  # column base of fout chunk
                bias_t = bpool.tile([128, 8 * L], FP16, tag="bias")
                for kt in range(8):
                    nc.sync.dma_start(
                        out=bias_t[:, kt * L : (kt + 1) * L],
                        in_=biasT_d[h, kt * 128 : (kt + 1) * 128, :],
                    )
                for b in range(BPC):
                    # scores (K=64 quadrant matmuls) -> exp -> bias multiply
                    ex_tiles = []
                    for kt in range(8):
                        ps = psA.tile([128, L], FP32, tag="pss")
                        for qc in range(2):
                            nc.tensor.matmul(
                                ps[:, qc * 512 : (qc + 1) * 512],
                                kT[b][
                                    hp : hp + 64, hc + kt * 128 : hc + (kt + 1) * 128
                                ],
                                qT[b][hp : hp + 64, hc + qc * 512 : hc + (qc + 1) * 512],
                                start=True,
                                stop=True,
                            )
                        es = espool.tile([128, L], FP16, tag="es")
                        nc.scalar.activation(es[:], ps[:], Exp)
                        ex = epool.tile([128, L], FP16, tag="ex")
                        eng = nc.vector if kt < DVE_MULT_KT else nc.gpsimd
                        eng.tensor_tensor(
                            ex[:], es[:], bias_t[:, kt * L : (kt + 1) * L], Mult
                        )
                        ex_tiles.append(ex)

                    # attn @ v_aug, vA stationary: U[0:64]=x^T, U[64]=denom
                    U = psU.tile([128, L], FP32, tag="u")
                    vA_v = vA[b][:].rearrange("p (t h w) -> p t h w", t=8, h=8)
                    for kt in range(8):
                        for qc in range(2):
                            nc.tensor.matmul(
                                U[0:65, qc * 512 : (qc + 1) * 512],
                                vA_v[:, kt, h, :],
                                ex_tiles[kt][:, qc * 512 : (qc + 1) * 512],
                                start=(kt == 0),
                                stop=(kt == 7),
                            )

                    # stage unnormalized x^T and the denominator row; the
                    # reciprocal is batched per-batch (DVE reciprocal costs
                    # ~6.4ns/column regardless of partition count, so one
                    # [8, L] reciprocal beats eight [1, L] ones 8x)
                    nc.vector.tensor_copy(
                        xatT[b][hp : hp + 64, hc : hc + L], U[0:64, :]
                    )
                    dr = (h % 4) * 32
                    dc = (h // 4) * L
                    nc.vector.tensor_copy(
                        dstage[b][dr : dr + 1, dc : dc + L], U[64:65, :]
                    )
                    if h == H - 1:
                        emit_norm_out(b)

    nc.compile()
    return nc


_NC = None


def _get_nc():
    global _NC
    if _NC is None:
        _NC = _build()
    return _NC


def _prep_in_maps(inputs):
    bf16 = ml_dtypes.bfloat16
    xq = np.asarray(inputs["inputs_q"], dtype=np.float32)
    xkv = np.asarray(inputs["inputs_kv"], dtype=np.float32)
    Wq = (np.asarray(inputs["Wq"], dtype=np.float32) * 0.125).astype(bf16)
    bq = np.asarray(inputs["bq"], dtype=np.float32) * 0.125
    Wk = np.asarray(inputs["Wk"], dtype=np.float32).astype(bf16)
    bk = np.asarray(inputs["bk"], dtype=np.float32)
    Wv = np.asarray(inputs["Wv"], dtype=np.float32).astype(bf16)
    bv_pad = np.zeros((128, F), dtype=np.float32)
    bv_pad[0] = np.asarray(inputs["bv"], dtype=np.float32)
    Wo = np.asarray(inputs["Wo"], dtype=np.float32).astype(bf16)
    bo_pad = np.zeros((128, F), dtype=np.float32)
    bo_pad[0] = np.asarray(inputs["bo"], dtype=np.float32)
    onesrow = np.zeros((128, 128), dtype=np.float32)
    onesrow[0] = 1.0
    toe = np.asarray(inputs["toeplitz"], dtype=np.float32)

    xqT = np.ascontiguousarray(xq.transpose(0, 2, 1)).astype(bf16)  # [B, F, L]
    xkvT = np.ascontiguousarray(xkv.transpose(0, 2, 1)).astype(bf16)

    coords = np.arange(L)
    xi, yi = coords // NY, coords % NY
    dx = xi[:, None] - xi[None, :] + NX
    dy = yi[:, None] - yi[None, :] + NY
    idx = dx * (2 * NY) + dy  # [L(q), L(k)]
    bias = toe[:, idx]  # [H, L(q), L(k)]
    biasT = np.exp(np.ascontiguousarray(bias.transpose(0, 2, 1))).astype(np.float16)

    in_maps = []
    for i in range(NCORES):
        sl = slice(i * BPC, (i + 1) * BPC)
        in_maps.append(
            {
                "xqT": np.ascontiguousarray(xqT[sl]),
                "xkvT": np.ascontiguousarray(xkvT[sl]),
                "Wq": Wq, "Wk": Wk, "Wv": Wv, "Wo": Wo,
                "bq": bq, "bk": bk, "bv": bv_pad, "bo": bo_pad,
                "biasT": biasT,
                "ones": onesrow,
            }
        )
    return in_maps


def _run(inputs, trace=False):
    from concourse.bass_interp import get_hw_module

    nc = _get_nc()
    in_maps = _prep_in_maps(inputs)
    old_m = nc.m
    nc.m = get_hw_module(nc.m)
    try:
        res = run_bass_kernel_spmd(
            nc, in_maps, core_ids=list(range(NCORES)), trace=trace
        )
    finally:
        nc.m = old_m
    out = np.concatenate([r["out"] for r in res.results], axis=0)  # [B, L, F]
    return out.reshape(B, L, H, D), res


def kernel(**inputs) -> np.ndarray:
    out, _ = _run(inputs, trace=False)
    return out


# revision 27
# speedup vs baseline: 1.6823x; 1.0924x over previous
"""Trainium2 Bass kernel: multi-head attention with Toeplitz relative bias.

Problem: B=16, L=1024, F=512, H=8, D=64 ViT patch attention.
Sharding: data-parallel over batch, 2 batches per core across 8 cores.

Device-side design (per core, fully unrolled Tile program):
  - Host pre-transposes inputs to xT [F, L] (bf16); 1/sqrt(D) folded into Wq.
  - qT/kT computed transposed ([fout, L]) in natural head-pair layout (head h
    on partitions (h%2)*64..+64 of fout chunk h//2). No zero padding: the
    scores matmul runs K=64 with both operands at partition base (h%2)*64
    (PE quadrant addressing).
  - v computed natural [L, fout] and packed as vA [128, kt, h, 65] fp16 with
    a ones column at index 64 (bv folded in via a ones-row matmul).
  - Scores [k, q] per (b, h, kt) in PSUM; ACT does exp (the only ACT work:
    ACT is the pipeline floor at ~143us); DVE/Pool multiply the host-exp'd
    Toeplitz bias in fp16 (DVE 2x mode; split to keep both engines even).
  - attn@v with vA stationary: U[0:64] = unnormalized x^T, U[64] = softmax
    denominator, accumulated over the 8 kt chunks in PSUM. Streams 512-col
    moving operands so ldweights (~97ns fixed) hide behind 213ns streams
    (the old ex-stationary design paid ~100us of exposed ldweights).
  - Normalization: rc = 1/denom (DVE divide), partition_broadcast to 64 rows
    (GpSimd Q7), fused multiply+cast -> xatT bf16 (DVE), writing odd heads at
    partition base 64 so the output projection gets K=128 chunks.
  - Output projection from xatT (+bo via ones-row matmul), Pool copies PSUM
    ->SBUF, DMA out.
  - No max-subtraction in softmax: |scores| <~ 2 by construction.
"""

import sys

import numpy as np

for _p in ("/opt/trn_rl_repo",):
    if _p not in sys.path:
        sys.path.insert(0, _p)

import ml_dtypes

import concourse.bass as bass
import concourse.mybir as mybir
import concourse.tile as tile
from concourse import bacc
from concourse.bass_utils import run_bass_kernel_spmd

B, L, F, H, D = 16, 1024, 512, 8, 64
NX, NY = 32, 32
NCORES = 8
BPC = B // NCORES  # batches per core
FP32 = mybir.dt.float32
F32R = mybir.dt.float32r
BF16 = mybir.dt.bfloat16
FP16 = mybir.dt.float16
Exp = mybir.ActivationFunctionType.Exp
Add = mybir.AluOpType.add
Mult = mybir.AluOpType.mult
Div = mybir.AluOpType.divide

# how many of the 8 bias-multiply kt-tiles per (b,h) go to DVE (rest: Pool).
# GPSIMD/Pool cannot touch PSUM and runs elementwise ~3x slower than DVE
# (0.42 efficiency Q7 ucode); the chip also DVFS-throttles on total engine
# activity, so wasteful Pool work hurts twice. Pool only gets the deferred
# norm multiplies (SBUF-only, off the critical path).
DVE_MULT_KT = 8


def _build():
    nc = bacc.Bacc("TRN2", target_bir_lowering=False, debug=False)

    xqT_d = nc.dram_tensor("xqT", [BPC, F, L], BF16, kind="ExternalInput").ap()
    xkvT_d = nc.dram_tensor("xkvT", [BPC, F, L], BF16, kind="ExternalInput").ap()
    Wq_d = nc.dram_tensor("Wq", [F, F], BF16, kind="ExternalInput").ap()
    Wk_d = nc.dram_tensor("Wk", [F, F], BF16, kind="ExternalInput").ap()
    Wv_d = nc.dram_tensor("Wv", [F, F], BF16, kind="ExternalInput").ap()
    Wo_d = nc.dram_tensor("Wo", [F, F], BF16, kind="ExternalInput").ap()
    bq_d = nc.dram_tensor("bq", [F], FP32, kind="ExternalInput").ap()
    bk_d = nc.dram_tensor("bk", [F], FP32, kind="ExternalInput").ap()
    bv_d = nc.dram_tensor("bv", [128, F], F32R, kind="ExternalInput").ap()
    bo_d = nc.dram_tensor("bo", [128, F], F32R, kind="ExternalInput").ap()
    biasT_d = nc.dram_tensor("biasT", [H, L, L], FP16, kind="ExternalInput").ap()
    ones_d = nc.dram_tensor("ones", [128, 128], F32R, kind="ExternalInput").ap()
    out_d = nc.dram_tensor("out", [BPC, L, F], FP32, kind="ExternalOutput").ap()
    # DRAM scratch for the reciprocal rows: partition-broadcast DMA reads
    # require a DRAM source (SBUF sources need nonzero partition step)
    rcd_d = nc.dram_tensor("rcd", [BPC, 8, L], FP16, kind="Internal").ap()

    with tile.TileContext(nc) as tc:
        with (
            tc.tile_pool(name="const", bufs=1) as cpool,
            tc.tile_pool(name="xin", bufs=2) as xpool,
            tc.tile_pool(name="qkv", bufs=2) as qpool,
            tc.tile_pool(name="bias", bufs=2) as bpool,
            tc.tile_pool(name="es", bufs=3) as espool,
            tc.tile_pool(name="exq", bufs=5) as epool,
            tc.tile_pool(name="nrm", bufs=1) as npool,
            tc.tile_pool(name="os", bufs=2) as opool,
            tc.tile_pool(name="psA", bufs=2, space="PSUM") as psA,
            tc.tile_pool(name="psU", bufs=1, space="PSUM") as psU,
        ):
            # ---- constants ----
            Wv_s = cpool.tile([128, 4 * F], BF16, tag="Wv")
            Wq_s = cpool.tile([128, 4 * F], BF16, tag="Wq")
            Wk_s = cpool.tile([128, 4 * F], BF16, tag="Wk")
            Wo_s = cpool.tile([128, 4 * F], BF16, tag="Wo")

            def load_w(w_s, w_d):
                nc.sync.dma_start(
                    out=w_s[:].rearrange("p (c n) -> p c n", c=4),
                    in_=w_d.rearrange("(c p) n -> p c n", c=4),
                )

            for kc in range(4):  # stream Wv first so v-proj starts ASAP
                nc.sync.dma_start(
                    out=Wv_s[:, kc * F : (kc + 1) * F],
                    in_=Wv_d[kc * 128 : (kc + 1) * 128, :],
                )
            ones_s = cpool.tile([128, 128], F32R, tag="ones")
            nc.sync.dma_start(out=ones_s[:], in_=ones_d)
            bv_s = cpool.tile([128, F], F32R, tag="bv")
            nc.sync.dma_start(out=bv_s[:], in_=bv_d)

            # ---- per-batch tiles ----
            qT, kT, vA, xatT, xq, xkv, dstage = [], [], [], [], [], [], []
            for b in range(BPC):
                xq.append(xpool.tile([128, 4 * L], BF16, tag="xq", name=f"xq{b}"))
                xkv.append(xpool.tile([128, 4 * L], BF16, tag="xkv", name=f"xkv{b}"))
                qT.append(qpool.tile([128, 4 * L], BF16, tag="qT", name=f"qT{b}"))
                kT.append(qpool.tile([128, 4 * L], BF16, tag="kT", name=f"kT{b}"))
                vA.append(
                    qpool.tile([128, 8 * 8 * 65], FP16, tag="vA", name=f"vA{b}")
                )
                xatT.append(
                    qpool.tile([128, 4 * L], BF16, tag="xatT", name=f"xatT{b}")
                )
                dstage.append(
                    qpool.tile([128, 2 * L], FP16, tag="dstage", name=f"dstage{b}")
                )

            # ---- phase A: load inputs + v projection ----
            for b in range(BPC):
                for lq in range(4):
                    nc.sync.dma_start(
                        out=xkv[b][:]
                        .rearrange("p (c l) -> p c l", c=4)[
                            :, :, lq * 256 : (lq + 1) * 256
                        ],
                        in_=xkvT_d[b].rearrange("(c p) l -> p c l", c=4)[
                            :, :, lq * 256 : (lq + 1) * 256
                        ],
                    )
                if b == 0:
                    load_w(Wq_s, Wq_d)
                    load_w(Wk_s, Wk_d)
                    bq_s = cpool.tile([128, 4], FP32, tag="bq")
                    bk_s = cpool.tile([128, 4], FP32, tag="bk")
                    for b_s, b_d in ((bq_s, bq_d), (bk_s, bk_d)):
                        nc.sync.dma_start(
                            out=b_s[:], in_=b_d.rearrange("(c p) -> p c", p=128)
                        )
                    load_w(Wo_s, Wo_d)
                    bo_s = cpool.tile([128, F], F32R, tag="bo")
                    nc.sync.dma_start(out=bo_s[:], in_=bo_d)

                # v natural (+bv via ones-row matmul): xT stationary, Wv moving
                vA_v = vA[b][:].rearrange("p (t h w) -> p t h w", t=8, h=8)
                for lt in range(8):
                    pv = psA.tile([128, 512], FP32, tag="psv")
                    for kc in range(4):
                        nc.tensor.matmul(
                            pv[:],
                            xkv[b][:, kc * L + lt * 128 : kc * L + (lt + 1) * 128],
                            Wv_s[:, kc * F : (kc + 1) * F],
                            start=(kc == 0),
                            stop=False,
                        )
                    nc.tensor.matmul(pv[:], ones_s[:], bv_s[:], start=False, stop=True)
                    nc.scalar.copy(
                        vA_v[:, lt, :, 0:64],
                        pv[:].rearrange("p (h w) -> p h w", h=8),
                    )
                nc.gpsimd.memset(vA_v[:, :, :, 64:65], 1.0)
                nc.sync.dma_start(
                    out=xq[b][:].rearrange("p (c l) -> p c l", c=4),
                    in_=xqT_d[b].rearrange("(c p) l -> p c l", c=4),
                )

            def qk_proj(fo):
                # q/k projections for fout chunk fo (= head pair 2fo, 2fo+1),
                # emitted just-in-time before the heads that consume them.
                for b in range(BPC):
                    for which, w_s, b_s, x_t, dst in (
                        ("q", Wq_s, bq_s, xq[b], qT[b]),
                        ("k", Wk_s, bk_s, xkv[b], kT[b]),
                    ):
                        for lc in range(2):
                            pq = psA.tile([128, 512], FP32, tag="psv")
                            for kc in range(4):
                                nc.tensor.matmul(
                                    pq[:],
                                    w_s[:, kc * F + fo * 128 : kc * F + (fo + 1) * 128],
                                    x_t[:, kc * L + lc * 512 : kc * L + (lc + 1) * 512],
                                    start=(kc == 0),
                                    stop=(kc == 3),
                                )
                            nc.vector.tensor_scalar(
                                dst[:, fo * L + lc * 512 : fo * L + (lc + 1) * 512],
                                pq[:],
                                b_s[:, fo : fo + 1],
                                None,
                                op0=Add,
                            )

            def emit_norm_out(b):
                # batched softmax normalization. Engine partition starts are
                # restricted to {0,32,64,96}, so: DMA-repack the 8 quadrant
                # denominator rows to 8 contiguous rows, ONE [8, L] reciprocal
                # (DVE reciprocal costs ~6.4ns/column regardless of partition
                # count), cast to fp16, DMA-spread back to quadrant rows for
                # the GpSimd broadcasts, then in-place multiplies (DVE 2x).
                d8 = npool.tile([8, L], FP16, tag="d8")
                nc.sync.dma_start(
                    out=d8[:],
                    in_=dstage[b][:].rearrange("p (c l) -> p c l", c=2)[
                        0:97:32, :, :
                    ],
                )
                rc32 = npool.tile([8, L], FP32, tag="rc32")
                nc.vector.reciprocal(rc32[:], d8[:])
                rc16 = npool.tile([8, L], FP16, tag="rc16")
                nc.vector.tensor_copy(rc16[:], rc32[:])
                nc.sync.dma_start(out=rcd_d[b], in_=rc16[:])
                for h in range(H):
                    hp = (h % 2) * 64
                    hc = (h // 2) * L
                    # broadcast 1/denom to 64 rows via DMA from DRAM (stride-0
                    # partition source); land at xatT's partition base so the
                    # two SBUF inputs of tensor_tensor share a base.
                    # d8 pack order is partition-major: row = (h%4)*2 + h//4
                    hr = (h % 4) * 2 + h // 4
                    rcb = npool.tile([128, L], FP16, tag="rcb", bufs=2)
                    nc.sync.dma_start(
                        out=rcb[hp : hp + 64, :],
                        in_=rcd_d[b][hr : hr + 1, :].to_broadcast((64, L)),
                    )
                    nc.gpsimd.tensor_tensor(
                        xatT[b][hp : hp + 64, hc : hc + L],
                        xatT[b][hp : hp + 64, hc : hc + L],
                        rcb[hp : hp + 64, :],
                        Mult,
                    )
                    del rcb
                # output projection (+bo via ones-row matmul)
                for lt in range(8):
                    po = psA.tile([128, 512], FP32, tag="psv")
                    for c in range(4):
                        nc.tensor.matmul(
                            po[:],
                            xatT[b][:, c * L + lt * 128 : c * L + (lt + 1) * 128],
                            Wo_s[:, c * F : (c + 1) * F],
                            start=(c == 0),
                            stop=False,
                        )
                    nc.tensor.matmul(po[:], ones_s[:], bo_s[:], start=False, stop=True)
                    os_t = opool.tile([128, 512], FP32, tag="os")
                    nc.scalar.copy(os_t[:], po[:])
                    nc.sync.dma_start(
                        out=out_d[b, lt * 128 : (lt + 1) * 128, :], in_=os_t[:]
                    )

            # ---- phase B: attention head loop ----
            for h in range(H):
                if h % 2 == 0:
                    qk_proj(h // 2)
                hp = (h % 2) * 64  # partition base within fout chunk
                hc = (h // 2) * L  # column base of fout chunk
                bias_t = bpool.tile([128, 8 * L], FP16, tag="bias")
                for kt in range(8):
                    nc.sync.dma_start(
                        out=bias_t[:, kt * L : (kt + 1) * L],
                        in_=biasT_d[h, kt * 128 : (kt + 1) * 128, :],
                    )
                for b in range(BPC):
                    # scores (K=64 quadrant matmuls) -> exp -> bias multiply
                    ex_tiles = []
                    for kt in range(8):
                        ps = psA.tile([128, L], FP32, tag="pss")
                        for qc in range(2):
                            nc.tensor.matmul(
                                ps[:, qc * 512 : (qc + 1) * 512],
                                kT[b][
                                    hp : hp + 64, hc + kt * 128 : hc + (kt + 1) * 128
                                ],
                                qT[b][hp : hp + 64, hc + qc * 512 : hc + (qc + 1) * 512],
                                start=True,
                                stop=True,
                            )
                        es = espool.tile([128, L], FP16, tag="es")
                        nc.scalar.activation(es[:], ps[:], Exp)
                        ex = epool.tile([128, L], FP16, tag="ex")
                        eng = nc.vector if kt < DVE_MULT_KT else nc.gpsimd
                        eng.tensor_tensor(
                            ex[:], es[:], bias_t[:, kt * L : (kt + 1) * L], Mult
                        )
                        ex_tiles.append(ex)

                    # attn @ v_aug, vA stationary: U[0:64]=x^T, U[64]=denom
                    U = psU.tile([128, L], FP32, tag="u")
                    vA_v = vA[b][:].rearrange("p (t h w) -> p t h w", t=8, h=8)
                    for kt in range(8):
                        for qc in range(2):
                            nc.tensor.matmul(
                                U[0:65, qc * 512 : (qc + 1) * 512],
                                vA_v[:, kt, h, :],
                                ex_tiles[kt][:, qc * 512 : (qc + 1) * 512],
                                start=(kt == 0),
                                stop=(kt == 7),
                            )

                    # stage unnormalized x^T and the denominator row; the
                    # reciprocal is batched per-batch (DVE reciprocal costs
                    # ~6.4ns/column regardless of partition count, so one
                    # [8, L] reciprocal beats eight [1, L] ones 8x)
                    nc.vector.tensor_copy(
                        xatT[b][hp : hp + 64, hc : hc + L], U[0:64, :]
                    )
                    dr = (h % 4) * 32
                    dc = (h // 4) * L
                    nc.vector.tensor_copy(
                        dstage[b][dr : dr + 1, dc : dc + L], U[64:65, :]
                    )
                # emit the normalize+output-projection for both batches only
                # after both have finished their last head, so b1's PE work
                # hides b0's norm chain and b0's outproj hides b1's.
                if h == H - 1:
                    for b in range(BPC):
                        emit_norm_out(b)

    nc.compile()
    return nc


_NC = None


def _get_nc():
    global _NC
    if _NC is None:
        _NC = _build()
    return _NC


def _prep_in_maps(inputs):
    bf16 = ml_dtypes.bfloat16
    xq = np.asarray(inputs["inputs_q"], dtype=np.float32)
    xkv = np.asarray(inputs["inputs_kv"], dtype=np.float32)
    Wq = (np.asarray(inputs["Wq"], dtype=np.float32) * 0.125).astype(bf16)
    bq = np.asarray(inputs["bq"], dtype=np.float32) * 0.125
    Wk = np.asarray(inputs["Wk"], dtype=np.float32).astype(bf16)
    bk = np.asarray(inputs["bk"], dtype=np.float32)
    Wv = np.asarray(inputs["Wv"], dtype=np.float32).astype(bf16)
    bv_pad = np.zeros((128, F), dtype=np.float32)
    bv_pad[0] = np.asarray(inputs["bv"], dtype=np.float32)
    Wo = np.asarray(inputs["Wo"], dtype=np.float32).astype(bf16)
    bo_pad = np.zeros((128, F), dtype=np.float32)
    bo_pad[0] = np.asarray(inputs["bo"], dtype=np.float32)
    onesrow = np.zeros((128, 128), dtype=np.float32)
    onesrow[0] = 1.0
    toe = np.asarray(inputs["toeplitz"], dtype=np.float32)

    xqT = np.ascontiguousarray(xq.transpose(0, 2, 1)).astype(bf16)  # [B, F, L]
    xkvT = np.ascontiguousarray(xkv.transpose(0, 2, 1)).astype(bf16)

    coords = np.arange(L)
    xi, yi = coords // NY, coords % NY
    dx = xi[:, None] - xi[None, :] + NX
    dy = yi[:, None] - yi[None, :] + NY
    idx = dx * (2 * NY) + dy  # [L(q), L(k)]
    bias = toe[:, idx]  # [H, L(q), L(k)]
    biasT = np.exp(np.ascontiguousarray(bias.transpose(0, 2, 1))).astype(np.float16)

    in_maps = []
    for i in range(NCORES):
        sl = slice(i * BPC, (i + 1) * BPC)
        in_maps.append(
            {
                "xqT": np.ascontiguousarray(xqT[sl]),
                "xkvT": np.ascontiguousarray(xkvT[sl]),
                "Wq": Wq, "Wk": Wk, "Wv": Wv, "Wo": Wo,
                "bq": bq, "bk": bk, "bv": bv_pad, "bo": bo_pad,
                "biasT": biasT,
                "ones": onesrow,
            }
        )
    return in_maps


def _run(inputs, trace=False):
    from concourse.bass_interp import get_hw_module

    nc = _get_nc()
    in_maps = _prep_in_maps(inputs)
    old_m = nc.m
    nc.m = get_hw_module(nc.m)
    try:
        res = run_bass_kernel_spmd(
            nc, in_maps, core_ids=list(range(NCORES)), trace=trace
        )
    finally:
        nc.m = old_m
    out = np.concatenate([r["out"] for r in res.results], axis=0)  # [B, L, F]
    return out.reshape(B, L, H, D), res


def kernel(**inputs) -> np.ndarray:
    out, _ = _run(inputs, trace=False)
    return out


# revision 30
# speedup vs baseline: 1.6962x; 1.0082x over previous
"""Trainium2 Bass kernel: multi-head attention with Toeplitz relative bias.

Problem: B=16, L=1024, F=512, H=8, D=64 ViT patch attention.
Sharding: data-parallel over batch, 2 batches per core across 8 cores.

Device-side design (per core, fully unrolled Tile program):
  - Host pre-transposes inputs to xT [F, L] (bf16); 1/sqrt(D) folded into Wq.
  - qT/kT computed transposed ([fout, L]) in natural head-pair layout (head h
    on partitions (h%2)*64..+64 of fout chunk h//2). No zero padding: the
    scores matmul runs K=64 with both operands at partition base (h%2)*64
    (PE quadrant addressing).
  - v computed natural [L, fout] and packed as vA [128, kt, h, 65] fp16 with
    a ones column at index 64 (bv folded in via a ones-row matmul).
  - Scores [k, q] per (b, h, kt) in PSUM; ACT does exp (the only ACT work:
    ACT is the pipeline floor at ~143us); DVE/Pool multiply the host-exp'd
    Toeplitz bias in fp16 (DVE 2x mode; split to keep both engines even).
  - attn@v with vA stationary: U[0:64] = unnormalized x^T, U[64] = softmax
    denominator, accumulated over the 8 kt chunks in PSUM. Streams 512-col
    moving operands so ldweights (~97ns fixed) hide behind 213ns streams
    (the old ex-stationary design paid ~100us of exposed ldweights).
  - Normalization: rc = 1/denom (DVE divide), partition_broadcast to 64 rows
    (GpSimd Q7), fused multiply+cast -> xatT bf16 (DVE), writing odd heads at
    partition base 64 so the output projection gets K=128 chunks.
  - Output projection from xatT (+bo via ones-row matmul), Pool copies PSUM
    ->SBUF, DMA out.
  - No max-subtraction in softmax: |scores| <~ 2 by construction.
"""

import sys

import numpy as np

for _p in ("/opt/trn_rl_repo",):
    if _p not in sys.path:
        sys.path.insert(0, _p)

import ml_dtypes

import concourse.bass as bass
import concourse.mybir as mybir
import concourse.tile as tile
from concourse import bacc
from concourse.bass_utils import run_bass_kernel_spmd

B, L, F, H, D = 16, 1024, 512, 8, 64
NX, NY = 32, 32
NCORES = 8
BPC = B // NCORES  # batches per core
FP32 = mybir.dt.float32
F32R = mybir.dt.float32r
BF16 = mybir.dt.bfloat16
FP16 = mybir.dt.float16
Exp = mybir.ActivationFunctionType.Exp
Add = mybir.AluOpType.add
Mult = mybir.AluOpType.mult
Div = mybir.AluOpType.divide

# how many of the 8 bias-multiply kt-tiles per (b,h) go to DVE (rest: Pool).
# GPSIMD/Pool cannot touch PSUM and runs elementwise ~3x slower than DVE
# (0.42 efficiency Q7 ucode); the chip also DVFS-throttles on total engine
# activity, so wasteful Pool work hurts twice. Pool only gets the deferred
# norm multiplies (SBUF-only, off the critical path).
DVE_MULT_KT = 8


def _build():
    nc = bacc.Bacc("TRN2", target_bir_lowering=False, debug=False)

    xqT_d = nc.dram_tensor("xqT", [BPC, F, L], BF16, kind="ExternalInput").ap()
    xkvT_d = nc.dram_tensor("xkvT", [BPC, F, L], BF16, kind="ExternalInput").ap()
    Wq_d = nc.dram_tensor("Wq", [F, F], BF16, kind="ExternalInput").ap()
    Wk_d = nc.dram_tensor("Wk", [F, F], BF16, kind="ExternalInput").ap()
    Wv_d = nc.dram_tensor("Wv", [F, F], BF16, kind="ExternalInput").ap()
    Wo_d = nc.dram_tensor("Wo", [F, F], BF16, kind="ExternalInput").ap()
    bq_d = nc.dram_tensor("bq", [F], FP32, kind="ExternalInput").ap()
    bk_d = nc.dram_tensor("bk", [F], FP32, kind="ExternalInput").ap()
    bv_d = nc.dram_tensor("bv", [128, F], F32R, kind="ExternalInput").ap()
    bo_d = nc.dram_tensor("bo", [128, F], F32R, kind="ExternalInput").ap()
    biasT_d = nc.dram_tensor("biasT", [H, L, L], FP16, kind="ExternalInput").ap()
    ones_d = nc.dram_tensor("ones", [128, 128], F32R, kind="ExternalInput").ap()
    out_d = nc.dram_tensor("out", [BPC, L, F], FP32, kind="ExternalOutput").ap()
    # DRAM scratch for the reciprocal rows: partition-broadcast DMA reads
    # require a DRAM source (SBUF sources need nonzero partition step)
    rcd_d = nc.dram_tensor("rcd", [BPC, 8, L], FP16, kind="Internal").ap()

    with tile.TileContext(nc) as tc:
        with (
            tc.tile_pool(name="const", bufs=1) as cpool,
            tc.tile_pool(name="xin", bufs=2) as xpool,
            tc.tile_pool(name="qkv", bufs=2) as qpool,
            tc.tile_pool(name="bias", bufs=2) as bpool,
            tc.tile_pool(name="es", bufs=3) as espool,
            tc.tile_pool(name="exq", bufs=5) as epool,
            tc.tile_pool(name="nrm", bufs=1) as npool,
            tc.tile_pool(name="os", bufs=2) as opool,
            tc.tile_pool(name="psA", bufs=2, space="PSUM") as psA,
            tc.tile_pool(name="psU", bufs=1, space="PSUM") as psU,
        ):
            # ---- constants ----
            Wv_s = cpool.tile([128, 4 * F], BF16, tag="Wv")
            Wq_s = cpool.tile([128, 4 * F], BF16, tag="Wq")
            Wk_s = cpool.tile([128, 4 * F], BF16, tag="Wk")
            Wo_s = cpool.tile([128, 4 * F], BF16, tag="Wo")

            def load_w(w_s, w_d):
                nc.sync.dma_start(
                    out=w_s[:].rearrange("p (c n) -> p c n", c=4),
                    in_=w_d.rearrange("(c p) n -> p c n", c=4),
                )

            for kc in range(4):  # stream Wv first so v-proj starts ASAP
                nc.sync.dma_start(
                    out=Wv_s[:, kc * F : (kc + 1) * F],
                    in_=Wv_d[kc * 128 : (kc + 1) * 128, :],
                )
            ones_s = cpool.tile([128, 128], F32R, tag="ones")
            nc.sync.dma_start(out=ones_s[:], in_=ones_d)
            bv_s = cpool.tile([128, F], F32R, tag="bv")
            nc.sync.dma_start(out=bv_s[:], in_=bv_d)

            # ---- per-batch tiles ----
            qT, kT, vA, xatT, xq, xkv, dstage = [], [], [], [], [], [], []
            for b in range(BPC):
                xq.append(xpool.tile([128, 4 * L], BF16, tag="xq", name=f"xq{b}"))
                xkv.append(xpool.tile([128, 4 * L], BF16, tag="xkv", name=f"xkv{b}"))
                qT.append(qpool.tile([128, 4 * L], BF16, tag="qT", name=f"qT{b}"))
                kT.append(qpool.tile([128, 4 * L], BF16, tag="kT", name=f"kT{b}"))
                vA.append(
                    qpool.tile([128, 8 * 8 * 65], FP16, tag="vA", name=f"vA{b}")
                )
                xatT.append(
                    qpool.tile([128, 4 * L], BF16, tag="xatT", name=f"xatT{b}")
                )
                dstage.append(
                    qpool.tile([128, 2 * L], FP16, tag="dstage", name=f"dstage{b}")
                )

            # ---- phase A: load inputs + v projection ----
            for b in range(BPC):
                for lq in range(4):
                    nc.sync.dma_start(
                        out=xkv[b][:]
                        .rearrange("p (c l) -> p c l", c=4)[
                            :, :, lq * 256 : (lq + 1) * 256
                        ],
                        in_=xkvT_d[b].rearrange("(c p) l -> p c l", c=4)[
                            :, :, lq * 256 : (lq + 1) * 256
                        ],
                    )
                if b == 0:
                    load_w(Wq_s, Wq_d)
                    load_w(Wk_s, Wk_d)
                    bq_s = cpool.tile([128, 4], FP32, tag="bq")
                    bk_s = cpool.tile([128, 4], FP32, tag="bk")
                    for b_s, b_d in ((bq_s, bq_d), (bk_s, bk_d)):
                        nc.sync.dma_start(
                            out=b_s[:], in_=b_d.rearrange("(c p) -> p c", p=128)
                        )
                    load_w(Wo_s, Wo_d)
                    bo_s = cpool.tile([128, F], F32R, tag="bo")
                    nc.sync.dma_start(out=bo_s[:], in_=bo_d)

                # v natural (+bv via ones-row matmul): xT stationary, Wv moving
                vA_v = vA[b][:].rearrange("p (t h w) -> p t h w", t=8, h=8)
                for lt in range(8):
                    pv = psA.tile([128, 512], FP32, tag="psv")
                    for kc in range(4):
                        nc.tensor.matmul(
                            pv[:],
                            xkv[b][:, kc * L + lt * 128 : kc * L + (lt + 1) * 128],
                            Wv_s[:, kc * F : (kc + 1) * F],
                            start=(kc == 0),
                            stop=False,
                        )
                    nc.tensor.matmul(pv[:], ones_s[:], bv_s[:], start=False, stop=True)
                    nc.scalar.copy(
                        vA_v[:, lt, :, 0:64],
                        pv[:].rearrange("p (h w) -> p h w", h=8),
                    )
                nc.gpsimd.memset(vA_v[:, :, :, 64:65], 1.0)
                nc.sync.dma_start(
                    out=xq[b][:].rearrange("p (c l) -> p c l", c=4),
                    in_=xqT_d[b].rearrange("(c p) l -> p c l", c=4),
                )

            def qk_proj(fo):
                # q/k projections for fout chunk fo (= head pair 2fo, 2fo+1),
                # emitted just-in-time before the heads that consume them.
                for b in range(BPC):
                    for which, w_s, b_s, x_t, dst in (
                        ("q", Wq_s, bq_s, xq[b], qT[b]),
                        ("k", Wk_s, bk_s, xkv[b], kT[b]),
                    ):
                        for lc in range(2):
                            pq = psA.tile([128, 512], FP32, tag="psv")
                            for kc in range(4):
                                nc.tensor.matmul(
                                    pq[:],
                                    w_s[:, kc * F + fo * 128 : kc * F + (fo + 1) * 128],
                                    x_t[:, kc * L + lc * 512 : kc * L + (lc + 1) * 512],
                                    start=(kc == 0),
                                    stop=(kc == 3),
                                )
                            nc.vector.tensor_scalar(
                                dst[:, fo * L + lc * 512 : fo * L + (lc + 1) * 512],
                                pq[:],
                                b_s[:, fo : fo + 1],
                                None,
                                op0=Add,
                            )

            def emit_recip(b):
                # batched softmax normalization, stage 1. Engine partition
                # starts are restricted to {0,32,64,96}, so: DMA-repack the 8
                # quadrant denominator rows to 8 contiguous rows, ONE [8, L]
                # reciprocal (DVE reciprocal costs ~6.4ns/column regardless
                # of partition count), cast fp16, round-trip through DRAM for
                # the per-head partition-broadcast DMAs in stage 2.
                d8 = npool.tile([8, L], FP16, tag="d8", bufs=2, name=f"d8_{b}")
                nc.sync.dma_start(
                    out=d8[:],
                    in_=dstage[b][:].rearrange("p (c l) -> p c l", c=2)[
                        0:97:32, :, :
                    ],
                )
                rc32 = npool.tile([8, L], FP32, tag="rc32", bufs=2, name=f"rc32_{b}")
                nc.vector.reciprocal(rc32[:], d8[:])
                rc16 = npool.tile([8, L], FP16, tag="rc16", bufs=2, name=f"rc16_{b}")
                nc.vector.tensor_copy(rc16[:], rc32[:])
                nc.sync.dma_start(out=rcd_d[b], in_=rc16[:])

            def emit_norm_out(b):
                for h in range(H):
                    hp = (h % 2) * 64
                    hc = (h // 2) * L
                    # broadcast 1/denom to 64 rows via DMA from DRAM (stride-0
                    # partition source); land at xatT's partition base so the
                    # two SBUF inputs of tensor_tensor share a base.
                    # d8 pack order is partition-major: row = (h%4)*2 + h//4
                    hr = (h % 4) * 2 + h // 4
                    rcb = npool.tile([128, L], FP16, tag="rcb", bufs=2)
                    nc.sync.dma_start(
                        out=rcb[hp : hp + 64, :],
                        in_=rcd_d[b][hr : hr + 1, :].to_broadcast((64, L)),
                    )
                    nc.vector.tensor_tensor(
                        xatT[b][hp : hp + 64, hc : hc + L],
                        xatT[b][hp : hp + 64, hc : hc + L],
                        rcb[hp : hp + 64, :],
                        Mult,
                    )
                    del rcb
                # output projection (+bo via ones-row matmul)
                for lt in range(8):
                    po = psA.tile([128, 512], FP32, tag="psv")
                    for c in range(4):
                        nc.tensor.matmul(
                            po[:],
                            xatT[b][:, c * L + lt * 128 : c * L + (lt + 1) * 128],
                            Wo_s[:, c * F : (c + 1) * F],
                            start=(c == 0),
                            stop=False,
                        )
                    nc.tensor.matmul(po[:], ones_s[:], bo_s[:], start=False, stop=True)
                    os_t = opool.tile([128, 512], FP32, tag="os")
                    nc.scalar.copy(os_t[:], po[:])
                    nc.sync.dma_start(
                        out=out_d[b, lt * 128 : (lt + 1) * 128, :], in_=os_t[:]
                    )

            # ---- phase B: attention head loop ----
            for h in range(H):
                if h % 2 == 0:
                    qk_proj(h // 2)
                hp = (h % 2) * 64  # partition base within fout chunk
                hc = (h // 2) * L  # column base of fout chunk
                bias_t = bpool.tile([128, 8 * L], FP16, tag="bias")
                for kt in range(8):
                    nc.sync.dma_start(
                        out=bias_t[:, kt * L : (kt + 1) * L],
                        in_=biasT_d[h, kt * 128 : (kt + 1) * 128, :],
                    )
                for b in range(BPC):
                    # scores (K=64 quadrant matmuls) -> exp -> bias multiply
                    ex_tiles = []
                    for kt in range(8):
                        ps = psA.tile([128, L], FP32, tag="pss")
                        for qc in range(2):
                            nc.tensor.matmul(
                                ps[:, qc * 512 : (qc + 1) * 512],
                                kT[b][
                                    hp : hp + 64, hc + kt * 128 : hc + (kt + 1) * 128
                                ],
                                qT[b][hp : hp + 64, hc + qc * 512 : hc + (qc + 1) * 512],
                                start=True,
                                stop=True,
                            )
                        es = espool.tile([128, L], FP16, tag="es")
                        nc.scalar.activation(es[:], ps[:], Exp)
                        ex = epool.tile([128, L], FP16, tag="ex")
                        eng = nc.vector if kt < DVE_MULT_KT else nc.gpsimd
                        eng.tensor_tensor(
                            ex[:], es[:], bias_t[:, kt * L : (kt + 1) * L], Mult
                        )
                        ex_tiles.append(ex)

                    # attn @ v_aug, vA stationary: U[0:64]=x^T, U[64]=denom
                    U = psU.tile([128, L], FP32, tag="u")
                    vA_v = vA[b][:].rearrange("p (t h w) -> p t h w", t=8, h=8)
                    for kt in range(8):
                        for qc in range(2):
                            nc.tensor.matmul(
                                U[0:65, qc * 512 : (qc + 1) * 512],
                                vA_v[:, kt, h, :],
                                ex_tiles[kt][:, qc * 512 : (qc + 1) * 512],
                                start=(kt == 0),
                                stop=(kt == 7),
                            )

                    # stage unnormalized x^T and the denominator row; the
                    # reciprocal is batched per-batch (DVE reciprocal costs
                    # ~6.4ns/column regardless of partition count, so one
                    # [8, L] reciprocal beats eight [1, L] ones 8x)
                    # ACT drains U (it has slack between exps; DVE is busier)
                    nc.scalar.copy(xatT[b][hp : hp + 64, hc : hc + L], U[0:64, :])
                    dr = (h % 4) * 32
                    dc = (h // 4) * L
                    nc.vector.tensor_copy(
                        dstage[b][dr : dr + 1, dc : dc + L], U[64:65, :]
                    )
                    if h == H - 1:
                        emit_recip(b)
                # normalize-multiplies + output projection for both batches
                # after both finished their last head: b1's PE work hides
                # b0's norm chain and b0's outproj hides b1's multiplies.
                if h == H - 1:
                    for b in range(BPC):
                        emit_norm_out(b)

    nc.compile()
    return nc


_NC = None


def _get_nc():
    global _NC
    if _NC is None:
        _NC = _build()
    return _NC


def _prep_in_maps(inputs):
    bf16 = ml_dtypes.bfloat16
    xq = np.asarray(inputs["inputs_q"], dtype=np.float32)
    xkv = np.asarray(inputs["inputs_kv"], dtype=np.float32)
    Wq = (np.asarray(inputs["Wq"], dtype=np.float32) * 0.125).astype(bf16)
    bq = np.asarray(inputs["bq"], dtype=np.float32) * 0.125
    Wk = np.asarray(inputs["Wk"], dtype=np.float32).astype(bf16)
    bk = np.asarray(inputs["bk"], dtype=np.float32)
    Wv = np.asarray(inputs["Wv"], dtype=np.float32).astype(bf16)
    bv_pad = np.zeros((128, F), dtype=np.float32)
    bv_pad[0] = np.asarray(inputs["bv"], dtype=np.float32)
    Wo = np.asarray(inputs["Wo"], dtype=np.float32).astype(bf16)
    bo_pad = np.zeros((128, F), dtype=np.float32)
    bo_pad[0] = np.asarray(inputs["bo"], dtype=np.float32)
    onesrow = np.zeros((128, 128), dtype=np.float32)
    onesrow[0] = 1.0
    toe = np.asarray(inputs["toeplitz"], dtype=np.float32)

    xqT = np.ascontiguousarray(xq.transpose(0, 2, 1)).astype(bf16)  # [B, F, L]
    xkvT = np.ascontiguousarray(xkv.transpose(0, 2, 1)).astype(bf16)

    coords = np.arange(L)
    xi, yi = coords // NY, coords % NY
    dx = xi[:, None] - xi[None, :] + NX
    dy = yi[:, None] - yi[None, :] + NY
    idx = dx * (2 * NY) + dy  # [L(q), L(k)]
    bias = toe[:, idx]  # [H, L(q), L(k)]
    biasT = np.exp(np.ascontiguousarray(bias.transpose(0, 2, 1))).astype(np.float16)

    in_maps = []
    for i in range(NCORES):
        sl = slice(i * BPC, (i + 1) * BPC)
        in_maps.append(
            {
                "xqT": np.ascontiguousarray(xqT[sl]),
                "xkvT": np.ascontiguousarray(xkvT[sl]),
                "Wq": Wq, "Wk": Wk, "Wv": Wv, "Wo": Wo,
                "bq": bq, "bk": bk, "bv": bv_pad, "bo": bo_pad,
                "biasT": biasT,
                "ones": onesrow,
            }
        )
    return in_maps


def _run(inputs, trace=False):
    from concourse.bass_interp import get_hw_module

    nc = _get_nc()
    in_maps = _prep_in_maps(inputs)
    old_m = nc.m
    nc.m = get_hw_module(nc.m)
    try:
        res = run_bass_kernel_spmd(
            nc, in_maps, core_ids=list(range(NCORES)), trace=trace
        )
    finally:
        nc.m = old_m
    out = np.concatenate([r["out"] for r in res.results], axis=0)  # [B, L, F]
    return out.reshape(B, L, H, D), res


def kernel(**inputs) -> np.ndarray:
    out, _ = _run(inputs, trace=False)
    return out
